# revision 3
# baseline (speedup 1.0000x reference)
"""Multi-head attention (B=1, L=4096, D=512, H=8, DH=64) on 8 TRN2 NeuronCores.

Head-parallel: core h computes head h end-to-end, host reduces partial
y contributions (y_h = attn_h @ Wo[h*64:(h+1)*64, :]).

v4 — PE stream reduction + split exp + fine-grained post scheduling:
  - kq projection is ONE psum pass per i-tile (lhsT = [wq|wk] slab, w
    cols 64:192 of the [wq|wq|wk|wk|wv] layout): psum rows 0:64 = qT,
    rows 64:128 = kT. The row-offset duplicates that score-pairing
    needs come from two SBUF->SBUF partition-shift DMAs on the idle
    GpSimd ring instead of a second 512-col PE pass (i-tile 0 keeps the
    two-pass form). i-tile-0 groups are emitted LAST in each prologue
    iteration so the swap DMAs have cover.
  - exp of each [128,1024] score group is split by column range across
    ScalarE (table exp, 0:544) and VectorE (Schraudolph tensor_scalar,
    544:1024) so the stp psum bank pair drains in ~0.7us instead of
    1.1-1.2us; the score matmul recycling those banks two groups later
    stops stalling. 544/480 balances the engines' ns/col.
  - posts are cut into 9 small pieces pumped one-per-group (g>=6, when
    pv(i-1) is provably complete) so no single pumped piece injects
    >0.8us into an engine queue ahead of a latency-critical exp half,
    and no queued semaphore wait can park an engine: in-order engine
    queues suffer priority inversion otherwise.
  - output projection row-group-paired for i-tiles 0..6 (wo at
    partitions 0:64 AND 64:128; outT partition-shifted by GpSimd DMA):
    2 concurrent-pair streams instead of 4.
  - v-proj accumulates all 4 t-chunks in ONE psum tile, drained by a
    single batched copy (saves 3 DVE instruction overheads/tile).
  - last i-tile's posts chunked per 128 queries and pipelined across
    engines; y partials written fp16 (host reduces in f32).
  - All matmuls fp16. (fp8 DoubleRow: 2x MACs but the dual-fp8
    ldweights 64-col cap forces a second denominator matmul that
    exactly cancels the gain; DoublePixel measured no faster.)
  - Score tiles stay transposed (ST[j,i] = k_j.q_i) so P@V contracts
    over j with PT as the moving operand; per-query exp sums come from
    the ones-column 64 of vext.
"""

import os

import numpy as np

import concourse.bass as bass
import concourse.mybir as mybir
import concourse.tile as tile
from concourse import bacc
from concourse.bass import ts

F32 = mybir.dt.float32
F16 = mybir.dt.float16
I16 = mybir.dt.int16

L = 4096  # sequence length
D = 512  # model dim
H = 8  # heads
DH = 64  # head dim
P = 128  # partitions
DC = D // P  # d-chunks for the projection contraction (4)
IW = 512  # i-tile (query) width
NI = L // IW  # 8
NJ = L // P  # 32 j-tiles (key blocks)
GJ = 2  # j-tiles per exp group (2 PSUM banks per instruction)
NG = NJ // GJ  # groups per i-tile (16)
WCOLS = 320  # q-dup(128) + k-dup(128) + v(64)
N_CORES = 8
SPLIT = 544  # scalar/vector exp column split inside a [128,1024] group

LOG2E = 1.4426950408889634
A16 = 1024.0 * LOG2E * 0.125
B16 = 15360.0 - 44.7257

_CACHE = {}
LAST = {}


def build_bass():
    nc = bacc.Bacc(
        "TRN2", target_bir_lowering=False, debug=False, num_devices=N_CORES
    )
    xt = nc.dram_tensor("xt", [DC, P, L], F16, kind="ExternalInput")
    w = nc.dram_tensor("w", [DC, P, WCOLS], F16, kind="ExternalInput")
    wo = nc.dram_tensor("wo", [DH, D], F16, kind="ExternalInput")
    y = nc.dram_tensor("y", [L // P, P, D], F16, kind="ExternalOutput")

    with (
        tile.TileContext(nc) as tc,
        tc.tile_pool(name="const", bufs=1) as cpool,
        tc.tile_pool(name="ps", bufs=1, space="PSUM") as ppool,
        tc.tile_pool(name="pt", bufs=1) as pt_pool,
        tc.tile_pool(name="post", bufs=1) as post_pool,
        tc.tile_pool(name="yout", bufs=1) as yout_pool,
    ):
        x_sb = cpool.tile([P, DC, L], F16)
        w_sb = cpool.tile([P, DC, WCOLS], F16)
        wo2 = cpool.tile([P, D], F16)
        # first-needed first: w chunks and i-tile-0 x chunks interleaved
        # across the three DMA dispatch rings
        nc.sync.dma_start(w_sb[:, 0], w[0])
        nc.scalar.dma_start(w_sb[:, 1], w[1])
        nc.gpsimd.dma_start(w_sb[:, 2], w[2])
        nc.sync.dma_start(w_sb[:, 3], w[3])
        for c, e in zip(range(DC), (nc.gpsimd, nc.scalar, nc.gpsimd, nc.sync)):
            e.dma_start(x_sb[:, c, ts(0, IW)], xt[c, :, ts(0, IW)])
        nc.scalar.dma_start(wo2[0:DH], wo[:])
        nc.scalar.dma_start(wo2[DH:P], wo[:])
        for i in range(1, NI):
            nc.sync.dma_start(
                x_sb[:, :, ts(i, IW)],
                xt[:, :, ts(i, IW)].rearrange("c p l -> p c l"),
            )

        qdup = cpool.tile([P, L], F16)  # qT in rows 0:64 AND 64:128
        kdup = cpool.tile([P, L], F16)
        vext = cpool.tile([P, NJ, DH + 2], F16)
        nc.vector.memset(vext[:, :, DH], 1.0)
        # warm the ACT exp table while DMAs run
        warm = cpool.tile([1, 8], F32)
        nc.vector.memset(warm[:], 0.0)
        nc.scalar.activation(warm[:], warm[:], mybir.ActivationFunctionType.Exp)

        def emit_proj_kq(i2):
            if i2 == 0:
                # two full passes: no swap-DMA latency ahead of the very
                # first score groups. k first (it gates the j-tiles).
                for off, dst in ((P, kdup), (0, qdup)):
                    ps = ppool.tile([P, IW], F32, tag="proj", bufs=2, name="ps")
                    for c in range(DC):
                        nc.tensor.matmul(
                            ps[:],
                            lhsT=w_sb[:, c, off : off + P],
                            rhs=x_sb[:, c, ts(0, IW)],
                            start=(c == 0),
                            stop=(c == DC - 1),
                        )
                    nc.scalar.copy(dst[:, ts(0, IW)], ps[:])
                return
            # single pass: lhsT = [wq|wk] (w cols 64:192) -> psum rows
            # 0:64 = qT, rows 64:128 = kT; row-offset dups via DMA
            ps = ppool.tile([P, IW], F32, tag="proj", bufs=2, name="ps")
            for c in range(DC):
                nc.tensor.matmul(
                    ps[:],
                    lhsT=w_sb[:, c, DH : DH + P],
                    rhs=x_sb[:, c, ts(i2, IW)],
                    start=(c == 0),
                    stop=(c == DC - 1),
                )
            nc.scalar.copy(qdup[0:DH, ts(i2, IW)], ps[0:DH, :])
            nc.vector.tensor_copy(kdup[DH:P, ts(i2, IW)], ps[DH:P, :])
            nc.gpsimd.dma_start(qdup[DH:P, ts(i2, IW)], qdup[0:DH, ts(i2, IW)])
            nc.gpsimd.dma_start(kdup[0:DH, ts(i2, IW)], kdup[DH:P, ts(i2, IW)])

        def emit_proj_v(i2):
            # v in row layout: all 4 t-chunks accumulate in ONE psum tile,
            # drained by a single batched copy
            psv = ppool.tile([P, 4, DH], F32, tag="proj", bufs=2, name="psv")
            for t in range(4):
                for c in range(DC):
                    nc.tensor.matmul(
                        psv[:, t],
                        lhsT=x_sb[:, c, ts(4 * i2 + t, P)],
                        rhs=w_sb[:, c, 2 * P : 2 * P + DH],
                        start=(c == 0),
                        stop=(c == DC - 1),
                        skip_group_check=True,
                    )
            nc.vector.tensor_copy(vext[:, 4 * i2 : 4 * i2 + 4, 0:DH], psv[:])

        pvs = {}
        outTs = {}
        outT2s = {}
        # PV matmuls lag the score/exp emission by PV_LAG groups so the
        # in-order PE stream never parks on a PV that is waiting for its
        # exp: scores of the next groups issue first.
        PV_LAG = 4
        pv_q = []

        def flush_pv(limit):
            while len(pv_q) > limit:
                i, g, pt = pv_q.pop(0)
                for u in range(GJ):
                    jt = g * GJ + u
                    nc.tensor.matmul(
                        pvs[i][:],
                        lhsT=vext[:, jt, 0 : DH + 1],
                        rhs=pt[:, ts(u, IW)],
                        start=(jt == 0),
                        stop=(jt == NJ - 1),
                        skip_group_check=True,
                    )

        def emit_group(i, g):
            if g == 0:
                pvs[i] = ppool.tile(
                    [DH + 1, IW], F32, tag="acc", bufs=2, name=f"pv{i}"
                )
            stp = ppool.tile([P, GJ * IW], F32, tag="st", bufs=2, name="stp")
            for u in range(GJ):
                jt = g * GJ + u
                half = DH * (jt % 2)
                nc.tensor.matmul(
                    stp[:, ts(u, IW)],
                    lhsT=kdup[half : half + DH, ts(jt, P)],
                    rhs=qdup[half : half + DH, ts(i, IW)],
                    start=True,
                    stop=True,
                )
            pt = pt_pool.tile([P, GJ * IW], F16, tag="pt", bufs=20, name="pt")
            nc.scalar.activation(
                pt[:, 0:SPLIT],
                stp[:, 0:SPLIT],
                mybir.ActivationFunctionType.Exp,
                scale=0.125,
            )
            nc.vector.tensor_scalar(
                pt[:, SPLIT : GJ * IW].bitcast(I16),
                stp[:, SPLIT : GJ * IW],
                A16,
                B16,
                mybir.AluOpType.mult,
                mybir.AluOpType.add,
            )
            pv_q.append((i, g, pt))
            flush_pv(PV_LAG)

        # --- posts, cut into small per-engine pieces ---
        rcps = {}
        rbs = {}

        def post_rcp(i):
            pv = pvs[i]
            srow = post_pool.tile([1, IW], F32, tag="srow", bufs=2, name="srow")
            nc.scalar.copy(srow[:], pv[DH : DH + 1, :])
            rcp = post_pool.tile([1, IW], F32, tag="rcp1", bufs=2, name="rcp")
            nc.vector.reciprocal_approx_fast(rcp[:], srow[:])
            rcps[i] = rcp

        def post_bcast(i):
            rb = post_pool.tile([DH, IW], F32, tag="rb", bufs=2, name="rb")
            nc.gpsimd.partition_broadcast(rb[:], rcps[i][:])
            rbs[i] = rb

        def post_mul(i, h):
            if h == 0:
                outTs[i] = post_pool.tile(
                    [DH, IW], F16, tag="outT", bufs=2, name="outT"
                )
            sl = slice(h * (IW // 2), (h + 1) * (IW // 2))
            nc.vector.tensor_mul(
                outTs[i][:, sl], pvs[i][0:DH, sl], rbs[i][:, sl]
            )

        def post_outT2(i):
            # partition-shifted copy of outT for out-proj row-group pairing
            o2 = post_pool.tile([P, IW], F16, tag="outT2", bufs=2, name="outT2")
            nc.gpsimd.dma_start(o2[DH:P, :], outTs[i][:, :])
            outT2s[i] = o2

        def post_ymm(i, p):
            # paired out-proj: chunk 2p on PE rows 0:64, chunk 2p+1 on 64:128
            ya = ppool.tile([P, D], F32, tag="proj", bufs=2, name="ya")
            yb = ppool.tile([P, D], F32, tag="proj", bufs=2, name="yb")
            nc.tensor.matmul(
                ya[:],
                lhsT=outTs[i][:, ts(2 * p, P)],
                rhs=wo2[0:DH],
                start=True,
                stop=True,
            )
            nc.tensor.matmul(
                yb[:],
                lhsT=outT2s[i][DH:P, ts(2 * p + 1, P)],
                rhs=wo2[DH:P],
                start=True,
                stop=True,
            )
            ysa = yout_pool.tile([P, D], F16, tag="ysb", bufs=4, name="ysa")
            nc.scalar.copy(ysa[:], ya[:])
            nc.sync.dma_start(y[i * (IW // P) + 2 * p], ysa[:])
            return yb

        def post_yb(i, p, yb):
            ysb_ = yout_pool.tile([P, D], F16, tag="ysb", bufs=4, name="ysb")
            nc.vector.tensor_copy(ysb_[:], yb[:])
            nc.sync.dma_start(y[i * (IW // P) + 2 * p + 1], ysb_[:])

        def pend_posts(pending, i):
            ybs = {}

            def mk_ymm(p):
                def f():
                    ybs[p] = post_ymm(i, p)

                return f

            pending.append(lambda: post_rcp(i))
            pending.append(lambda: post_bcast(i))
            pending.append(lambda: post_mul(i, 0))
            pending.append(lambda: post_mul(i, 1))
            pending.append(lambda: post_outT2(i))
            pending.append(mk_ymm(0))
            pending.append(lambda: post_yb(i, 0, ybs[0]))
            pending.append(mk_ymm(1))
            pending.append(lambda: post_yb(i, 1, ybs[1]))

        def emit_post_tail(i):
            # last i-tile: per-128-query chunks pipelined across engines;
            # score psum banks are free now so yps alternates tags
            pv = pvs[i]
            outT = post_pool.tile([DH, IW], F16, tag="outT", bufs=2, name="outTt")
            for t in range(IW // P):
                srow = post_pool.tile([1, P], F32, tag="srowc", bufs=2, name="srowc")
                nc.scalar.copy(srow[:], pv[DH : DH + 1, ts(t, P)])
                rcp = post_pool.tile([1, P], F32, tag="rcpc", bufs=2, name="rcpc")
                nc.vector.reciprocal_approx_fast(rcp[:], srow[:])
                rb = post_pool.tile([DH, P], F32, tag="rbc", bufs=2, name="rbc")
                nc.gpsimd.partition_broadcast(rb[:], rcp[:])
                nc.vector.tensor_mul(outT[:, ts(t, P)], pv[0:DH, ts(t, P)], rb[:])
                yps = ppool.tile(
                    [P, D],
                    F32,
                    tag="proj" if t % 2 == 0 else "st",
                    bufs=2,
                    name="yps",
                )
                nc.tensor.matmul(
                    yps[:],
                    lhsT=outT[:, ts(t, P)],
                    rhs=wo2[0:DH],
                    start=True,
                    stop=True,
                )
                ysb_ = yout_pool.tile([P, D], F16, tag="ysb", bufs=4, name="ysbt")
                if t % 2 == 0:
                    nc.scalar.copy(ysb_[:], yps[:])
                else:
                    nc.vector.tensor_copy(ysb_[:], yps[:])
                nc.sync.dma_start(y[i * (IW // P) + t], ysb_[:])

        # --- prologue: projections interleaved with i-tiles 0..2.
        # i-tile-0 groups go LAST in each iteration so the kq swap DMAs
        # for proj(i2) have the older tiles' groups as cover. ---
        from collections import deque

        pending = deque()

        def pump():
            if pending:
                pending.popleft()()

        for i2 in range(NI):
            emit_proj_kq(i2)
            emit_proj_v(i2)
            if i2 > 0:
                emit_group(1, 2 * (i2 - 1))
                emit_group(1, 2 * (i2 - 1) + 1)
            if i2 > 1:
                emit_group(2, 2 * (i2 - 2))
                emit_group(2, 2 * (i2 - 2) + 1)
            emit_group(0, 2 * i2)
            emit_group(0, 2 * i2 + 1)
        emit_group(1, NG - 2)
        emit_group(1, NG - 1)
        for g in range(2 * (NI - 2), NG):
            emit_group(2, g)
        flush_pv(0)
        for i in (0, 1, 2):
            pend_posts(pending, i)
        for _ in range(9):  # drain i0's posts before steady state
            pump()
        # --- steady state ---
        for i in range(3, NI):
            for g in range(NG):
                emit_group(i, g)
                # posts for i-1 are pumped from g >= 6 so the semaphore
                # their first piece waits on (pv(i-1) complete, emitted at
                # g == PV_LAG - 1, executed ~g+1) is satisfied before the
                # piece enters an engine queue: a queued wait parks the
                # whole in-order queue including later exp halves.
                if g >= 6:
                    pump()
            if i < NI - 1:
                pend_posts(pending, i)
        flush_pv(0)
        while pending:
            pump()
        emit_post_tail(NI - 1)
    nc.compile()
    return nc


def _get_nc():
    if "nc" not in _CACHE:
        _CACHE["nc"] = build_bass()
    return _CACHE["nc"]


def _prep_in_maps(x, Wqkv, Wo):
    x = np.asarray(x, dtype=np.float32).reshape(L, D)
    Wqkv = np.asarray(Wqkv, dtype=np.float32)
    Wo = np.asarray(Wo, dtype=np.float32)
    xt = np.ascontiguousarray(x.T).reshape(DC, P, L).astype(np.float16)
    in_maps = []
    for h in range(N_CORES):
        wq = Wqkv[:, 0 * D + h * DH : 0 * D + (h + 1) * DH]
        wk = Wqkv[:, 1 * D + h * DH : 1 * D + (h + 1) * DH]
        wv = Wqkv[:, 2 * D + h * DH : 2 * D + (h + 1) * DH]
        cols = np.concatenate([wq, wq, wk, wk, wv], axis=1)  # [512, 320]
        w_dram = np.ascontiguousarray(cols).reshape(DC, P, WCOLS).astype(np.float16)
        wo_h = np.ascontiguousarray(Wo[h * DH : (h + 1) * DH, :]).astype(np.float16)
        in_maps.append({"xt": xt, "w": w_dram, "wo": wo_h})
    return in_maps


def kernel(x, Wqkv, Wo):
    from concourse import bass_utils

    # zero-egress container: artifact upload is impossible and only feeds
    # trace metadata — replace with a local marker.
    bass_utils.upload_artifacts = lambda tmpdir: f"local://{tmpdir}"

    nc = _get_nc()
    in_maps = _prep_in_maps(x, Wqkv, Wo)
    trace = bool(os.environ.get("KERNEL_TRACE"))
    res = bass_utils.run_bass_kernel_spmd(
        nc, in_maps, core_ids=list(range(N_CORES)), trace=trace
    )
    LAST["exec_time_ns"] = res.exec_time_ns
    LAST["trace"] = res.instructions_and_trace
    acc = np.zeros((L, D), np.float32)
    for r in res.results:
        acc += r["y"].reshape(L, D).astype(np.float32)
    return acc.reshape(1, L, D).astype(np.float32)


# revision 5
# speedup vs baseline: 1.0275x; 1.0275x over previous
"""Multi-head attention (B=1, L=4096, D=512, H=8, DH=64) on 8 TRN2 NeuronCores.

Head-parallel: core h computes head h end-to-end, host reduces partial
y contributions (y_h = attn_h @ Wo[h*64:(h+1)*64, :]).

v4 — PE stream reduction + split exp + fine-grained post scheduling:
  - kq projection is ONE psum pass per i-tile (lhsT = [wq|wk] slab, w
    cols 64:192 of the [wq|wq|wk|wk|wv] layout): psum rows 0:64 = qT,
    rows 64:128 = kT. The row-offset duplicates that score-pairing
    needs come from two SBUF->SBUF partition-shift DMAs on the idle
    GpSimd ring instead of a second 512-col PE pass (i-tile 0 keeps the
    two-pass form). i-tile-0 groups are emitted LAST in each prologue
    iteration so the swap DMAs have cover.
  - exp of each [128,1024] score group is split by column range across
    ScalarE (table exp, 0:544) and VectorE (Schraudolph tensor_scalar,
    544:1024) so the stp psum bank pair drains in ~0.7us instead of
    1.1-1.2us; the score matmul recycling those banks two groups later
    stops stalling. 544/480 balances the engines' ns/col.
  - posts are cut into 9 small pieces pumped one-per-group (g>=6, when
    pv(i-1) is provably complete) so no single pumped piece injects
    >0.8us into an engine queue ahead of a latency-critical exp half,
    and no queued semaphore wait can park an engine: in-order engine
    queues suffer priority inversion otherwise.
  - output projection row-group-paired for i-tiles 0..6 (wo at
    partitions 0:64 AND 64:128; outT partition-shifted by GpSimd DMA):
    2 concurrent-pair streams instead of 4.
  - v-proj accumulates all 4 t-chunks in ONE psum tile, drained by a
    single batched copy (saves 3 DVE instruction overheads/tile).
  - last i-tile's posts chunked per 128 queries and pipelined across
    engines; y partials written fp16 (host reduces in f32).
  - All matmuls fp16. (fp8 DoubleRow: 2x MACs but the dual-fp8
    ldweights 64-col cap forces a second denominator matmul that
    exactly cancels the gain; DoublePixel measured no faster.)
  - Score tiles stay transposed (ST[j,i] = k_j.q_i) so P@V contracts
    over j with PT as the moving operand; per-query exp sums come from
    the ones-column 64 of vext.
"""

import os

import numpy as np

import concourse.bass as bass
import concourse.mybir as mybir
import concourse.tile as tile
from concourse import bacc
from concourse.bass import ts

F32 = mybir.dt.float32
F16 = mybir.dt.float16
I16 = mybir.dt.int16

L = 4096  # sequence length
D = 512  # model dim
H = 8  # heads
DH = 64  # head dim
P = 128  # partitions
DC = D // P  # d-chunks for the projection contraction (4)
IW = 512  # i-tile (query) width
NI = L // IW  # 8
NJ = L // P  # 32 j-tiles (key blocks)
GJ = 2  # j-tiles per exp group (2 PSUM banks per instruction)
NG = NJ // GJ  # groups per i-tile (16)
WCOLS = 320  # q-dup(128) + k-dup(128) + v(64)
N_CORES = 8
SPLIT = 544  # scalar/vector exp column split inside a [128,1024] group

LOG2E = 1.4426950408889634
A16 = 1024.0 * LOG2E * 0.125
B16 = 15360.0 - 44.7257

_CACHE = {}
LAST = {}


def build_bass():
    nc = bacc.Bacc(
        "TRN2", target_bir_lowering=False, debug=False, num_devices=N_CORES
    )
    xt = nc.dram_tensor("xt", [DC, P, L], F16, kind="ExternalInput")
    w = nc.dram_tensor("w", [DC, P, WCOLS], F16, kind="ExternalInput")
    wo = nc.dram_tensor("wo", [DH, D], F16, kind="ExternalInput")
    y = nc.dram_tensor("y", [L // P, P, D], F16, kind="ExternalOutput")

    with (
        tile.TileContext(nc) as tc,
        tc.tile_pool(name="const", bufs=1) as cpool,
        tc.tile_pool(name="ps", bufs=1, space="PSUM") as ppool,
        tc.tile_pool(name="pt", bufs=1) as pt_pool,
        tc.tile_pool(name="post", bufs=1) as post_pool,
        tc.tile_pool(name="yout", bufs=1) as yout_pool,
    ):
        x_sb = cpool.tile([P, DC, L], F16)
        w_sb = cpool.tile([P, DC, WCOLS], F16)
        wo2 = cpool.tile([P, D], F16)
        # first-needed first: w chunks and i-tile-0 x chunks interleaved
        # across the three DMA dispatch rings
        nc.sync.dma_start(w_sb[:, 0], w[0])
        nc.scalar.dma_start(w_sb[:, 1], w[1])
        nc.gpsimd.dma_start(w_sb[:, 2], w[2])
        nc.sync.dma_start(w_sb[:, 3], w[3])
        for c, e in zip(range(DC), (nc.gpsimd, nc.scalar, nc.gpsimd, nc.sync)):
            e.dma_start(x_sb[:, c, ts(0, IW)], xt[c, :, ts(0, IW)])
        nc.scalar.dma_start(wo2[0:DH], wo[:])
        nc.scalar.dma_start(wo2[DH:P], wo[:])
        for i in range(1, NI):
            nc.sync.dma_start(
                x_sb[:, :, ts(i, IW)],
                xt[:, :, ts(i, IW)].rearrange("c p l -> p c l"),
            )

        qdup = cpool.tile([P, L], F16)  # qT in rows 0:64 AND 64:128
        kdup = cpool.tile([P, L], F16)
        vext = cpool.tile([P, NJ, DH + 2], F16)
        nc.vector.memset(vext[:, :, DH], 1.0)
        # warm the ACT exp table while DMAs run
        warm = cpool.tile([1, 8], F32)
        nc.vector.memset(warm[:], 0.0)
        nc.scalar.activation(warm[:], warm[:], mybir.ActivationFunctionType.Exp)

        def emit_proj_kq(i2):
            if i2 == 0:
                # two full passes: no swap-DMA latency ahead of the very
                # first score groups. k first (it gates the j-tiles).
                for off, dst in ((P, kdup), (0, qdup)):
                    ps = ppool.tile([P, IW], F32, tag="proj", bufs=2, name="ps")
                    for c in range(DC):
                        nc.tensor.matmul(
                            ps[:],
                            lhsT=w_sb[:, c, off : off + P],
                            rhs=x_sb[:, c, ts(0, IW)],
                            start=(c == 0),
                            stop=(c == DC - 1),
                        )
                    nc.scalar.copy(dst[:, ts(0, IW)], ps[:])
                return
            # single pass: lhsT = [wq|wk] (w cols 64:192) -> psum rows
            # 0:64 = qT, rows 64:128 = kT; row-offset dups via DMA
            ps = ppool.tile([P, IW], F32, tag="proj", bufs=2, name="ps")
            for c in range(DC):
                nc.tensor.matmul(
                    ps[:],
                    lhsT=w_sb[:, c, DH : DH + P],
                    rhs=x_sb[:, c, ts(i2, IW)],
                    start=(c == 0),
                    stop=(c == DC - 1),
                )
            nc.scalar.copy(qdup[0:DH, ts(i2, IW)], ps[0:DH, :])
            nc.vector.tensor_copy(kdup[DH:P, ts(i2, IW)], ps[DH:P, :])
            nc.gpsimd.dma_start(qdup[DH:P, ts(i2, IW)], qdup[0:DH, ts(i2, IW)])
            nc.gpsimd.dma_start(kdup[0:DH, ts(i2, IW)], kdup[DH:P, ts(i2, IW)])

        def emit_proj_v(i2):
            # v in row layout: all 4 t-chunks accumulate in ONE psum tile,
            # drained by a single batched copy
            psv = ppool.tile([P, 4, DH], F32, tag="proj", bufs=2, name="psv")
            for t in range(4):
                for c in range(DC):
                    nc.tensor.matmul(
                        psv[:, t],
                        lhsT=x_sb[:, c, ts(4 * i2 + t, P)],
                        rhs=w_sb[:, c, 2 * P : 2 * P + DH],
                        start=(c == 0),
                        stop=(c == DC - 1),
                        skip_group_check=True,
                    )
            nc.vector.tensor_copy(vext[:, 4 * i2 : 4 * i2 + 4, 0:DH], psv[:])

        pvs = {}
        outTs = {}
        outT2s = {}
        # PV matmuls lag the score/exp emission by PV_LAG groups so the
        # in-order PE stream never parks on a PV that is waiting for its
        # exp: scores of the next groups issue first.
        PV_LAG = 4
        pv_q = []

        def flush_pv(limit):
            while len(pv_q) > limit:
                i, g, pt = pv_q.pop(0)
                for u in range(GJ):
                    jt = g * GJ + u
                    nc.tensor.matmul(
                        pvs[i][:],
                        lhsT=vext[:, jt, 0 : DH + 1],
                        rhs=pt[:, ts(u, IW)],
                        start=(jt == 0),
                        stop=(jt == NJ - 1),
                        skip_group_check=True,
                    )

        def emit_group(i, g):
            if g == 0:
                pvs[i] = ppool.tile(
                    [DH + 1, IW], F32, tag="acc", bufs=2, name=f"pv{i}"
                )
            stp = ppool.tile([P, GJ * IW], F32, tag="st", bufs=2, name="stp")
            for u in range(GJ):
                jt = g * GJ + u
                half = DH * (jt % 2)
                nc.tensor.matmul(
                    stp[:, ts(u, IW)],
                    lhsT=kdup[half : half + DH, ts(jt, P)],
                    rhs=qdup[half : half + DH, ts(i, IW)],
                    start=True,
                    stop=True,
                )
            pt = pt_pool.tile([P, GJ * IW], F16, tag="pt", bufs=20, name="pt")
            nc.scalar.activation(
                pt[:, 0:SPLIT],
                stp[:, 0:SPLIT],
                mybir.ActivationFunctionType.Exp,
                scale=0.125,
            )
            nc.vector.tensor_scalar(
                pt[:, SPLIT : GJ * IW].bitcast(I16),
                stp[:, SPLIT : GJ * IW],
                A16,
                B16,
                mybir.AluOpType.mult,
                mybir.AluOpType.add,
            )
            pv_q.append((i, g, pt))
            flush_pv(PV_LAG)

        # --- posts, cut into small per-engine pieces ---
        rcps = {}
        rbs = {}

        def post_rcp(i):
            pv = pvs[i]
            srow = post_pool.tile([1, IW], F32, tag="srow", bufs=2, name="srow")
            nc.scalar.copy(srow[:], pv[DH : DH + 1, :])
            rcp = post_pool.tile([1, IW], F32, tag="rcp1", bufs=2, name="rcp")
            nc.vector.reciprocal_approx_fast(rcp[:], srow[:])
            rcps[i] = rcp

        def post_bcast(i):
            rb = post_pool.tile([DH, IW], F32, tag="rb", bufs=2, name="rb")
            nc.gpsimd.partition_broadcast(rb[:], rcps[i][:])
            rbs[i] = rb

        def post_mul(i, h):
            if h == 0:
                outTs[i] = post_pool.tile(
                    [DH, IW], F16, tag="outT", bufs=2, name="outT"
                )
            sl = slice(h * (IW // 2), (h + 1) * (IW // 2))
            nc.vector.tensor_mul(
                outTs[i][:, sl], pvs[i][0:DH, sl], rbs[i][:, sl]
            )

        def post_outT2(i):
            # partition-shifted copy of outT for out-proj row-group pairing
            o2 = post_pool.tile([P, IW], F16, tag="outT2", bufs=2, name="outT2")
            nc.gpsimd.dma_start(o2[DH:P, :], outTs[i][:, :])
            outT2s[i] = o2

        def post_ymm(i, p):
            # paired out-proj: chunk 2p on PE rows 0:64, chunk 2p+1 on 64:128
            ya = ppool.tile([P, D], F32, tag="proj", bufs=2, name="ya")
            yb = ppool.tile([P, D], F32, tag="proj", bufs=2, name="yb")
            nc.tensor.matmul(
                ya[:],
                lhsT=outTs[i][:, ts(2 * p, P)],
                rhs=wo2[0:DH],
                start=True,
                stop=True,
            )
            nc.tensor.matmul(
                yb[:],
                lhsT=outT2s[i][DH:P, ts(2 * p + 1, P)],
                rhs=wo2[DH:P],
                start=True,
                stop=True,
            )
            ysa = yout_pool.tile([P, D], F16, tag="ysb", bufs=4, name="ysa")
            nc.scalar.copy(ysa[:], ya[:])
            nc.sync.dma_start(y[i * (IW // P) + 2 * p], ysa[:])
            return yb

        def post_yb(i, p, yb):
            ysb_ = yout_pool.tile([P, D], F16, tag="ysb", bufs=4, name="ysb")
            nc.vector.tensor_copy(ysb_[:], yb[:])
            nc.sync.dma_start(y[i * (IW // P) + 2 * p + 1], ysb_[:])

        def pend_posts(pending, i):
            ybs = {}

            def mk_ymm(p):
                def f():
                    ybs[p] = post_ymm(i, p)

                return f

            pending.append(lambda: post_rcp(i))
            pending.append(lambda: post_bcast(i))
            pending.append(lambda: post_mul(i, 0))
            pending.append(lambda: post_mul(i, 1))
            pending.append(lambda: post_outT2(i))
            pending.append(mk_ymm(0))
            pending.append(lambda: post_yb(i, 0, ybs[0]))
            pending.append(mk_ymm(1))
            pending.append(lambda: post_yb(i, 1, ybs[1]))

        def emit_post_tail(i):
            # last i-tile: phase-ordered emission (all srows, all rcps, ...)
            # so each in-order engine queue holds every chunk's work for
            # phase k before phase k+1 — per-chunk interleaved emission
            # serializes chunks through the queues instead.
            pv = pvs[i]
            NT = IW // P
            outT = post_pool.tile([DH, IW], F16, tag="outT", bufs=2, name="outTt")
            srows, rcpc, rbc = [], [], []
            for t in range(NT):
                srow = post_pool.tile([1, P], F32, tag="srowc", bufs=4, name="srowc")
                nc.scalar.copy(srow[:], pv[DH : DH + 1, ts(t, P)])
                srows.append(srow)
            for t in range(NT):
                rcp = post_pool.tile([1, P], F32, tag="rcpc", bufs=4, name="rcpc")
                nc.vector.reciprocal_approx_fast(rcp[:], srows[t][:])
                rcpc.append(rcp)
            for t in range(NT):
                rb = post_pool.tile([DH, P], F32, tag="rbc", bufs=4, name="rbc")
                nc.gpsimd.partition_broadcast(rb[:], rcpc[t][:])
                rbc.append(rb)
            for t in range(NT):
                nc.vector.tensor_mul(
                    outT[:, ts(t, P)], pv[0:DH, ts(t, P)], rbc[t][:]
                )
            for t in range(NT):
                # score psum banks are free now so yps alternates tags
                yps = ppool.tile(
                    [P, D],
                    F32,
                    tag="proj" if t % 2 == 0 else "st",
                    bufs=2,
                    name="yps",
                )
                nc.tensor.matmul(
                    yps[:],
                    lhsT=outT[:, ts(t, P)],
                    rhs=wo2[0:DH],
                    start=True,
                    stop=True,
                )
                ysb_ = yout_pool.tile([P, D], F16, tag="ysb", bufs=4, name="ysbt")
                if t % 2 == 0:
                    nc.scalar.copy(ysb_[:], yps[:])
                else:
                    nc.vector.tensor_copy(ysb_[:], yps[:])
                nc.sync.dma_start(y[i * (IW // P) + t], ysb_[:])

        # --- prologue: projections interleaved with i-tiles 0..2.
        # i-tile-0 groups go LAST in each iteration so the kq swap DMAs
        # for proj(i2) have the older tiles' groups as cover. ---
        from collections import deque

        pending = deque()

        def pump():
            if pending:
                pending.popleft()()

        for i2 in range(NI):
            emit_proj_kq(i2)
            emit_proj_v(i2)
            if i2 > 0:
                emit_group(1, 2 * (i2 - 1))
                emit_group(1, 2 * (i2 - 1) + 1)
            if i2 > 1:
                emit_group(2, 2 * (i2 - 2))
                emit_group(2, 2 * (i2 - 2) + 1)
            emit_group(0, 2 * i2)
            emit_group(0, 2 * i2 + 1)
        # trailing groups, ordered so pv(0) is fully emitted early (the
        # pv flush lags by PV_LAG); tile-0 post pieces interleave so the
        # PE never parks on a whole post chain at the transition
        emit_group(1, NG - 2)
        emit_group(1, NG - 1)
        emit_group(2, 2 * (NI - 2))
        emit_group(2, 2 * (NI - 2) + 1)  # pv(0) fully emitted after this
        pend_posts(pending, 0)
        pump()
        emit_group(2, NG - 2)
        pump()
        emit_group(2, NG - 1)
        pump()
        flush_pv(0)
        for _ in range(6):  # rest of tile-0's posts
            pump()
        pend_posts(pending, 1)
        pend_posts(pending, 2)
        # --- steady state ---
        for i in range(3, NI):
            for g in range(NG):
                emit_group(i, g)
                # posts for earlier tiles are pumped once the semaphore
                # their first piece waits on (that tile's pv complete) is
                # satisfied before the piece enters an engine queue: a
                # queued wait parks the whole in-order queue including
                # later exp halves. Tile 3 drains the two-tile backlog
                # left by the prologue (18 pieces over 18 slots).
                if i == 3:
                    if g >= 2:
                        pump()
                    if g >= 12:
                        pump()
                elif g >= 6:
                    pump()
            if i < NI - 1:
                pend_posts(pending, i)
        flush_pv(0)
        while pending:
            pump()
        emit_post_tail(NI - 1)
    nc.compile()
    return nc


def _get_nc():
    if "nc" not in _CACHE:
        _CACHE["nc"] = build_bass()
    return _CACHE["nc"]


def _prep_in_maps(x, Wqkv, Wo):
    x = np.asarray(x, dtype=np.float32).reshape(L, D)
    Wqkv = np.asarray(Wqkv, dtype=np.float32)
    Wo = np.asarray(Wo, dtype=np.float32)
    xt = np.ascontiguousarray(x.T).reshape(DC, P, L).astype(np.float16)
    in_maps = []
    for h in range(N_CORES):
        wq = Wqkv[:, 0 * D + h * DH : 0 * D + (h + 1) * DH]
        wk = Wqkv[:, 1 * D + h * DH : 1 * D + (h + 1) * DH]
        wv = Wqkv[:, 2 * D + h * DH : 2 * D + (h + 1) * DH]
        cols = np.concatenate([wq, wq, wk, wk, wv], axis=1)  # [512, 320]
        w_dram = np.ascontiguousarray(cols).reshape(DC, P, WCOLS).astype(np.float16)
        wo_h = np.ascontiguousarray(Wo[h * DH : (h + 1) * DH, :]).astype(np.float16)
        in_maps.append({"xt": xt, "w": w_dram, "wo": wo_h})
    return in_maps


def kernel(x, Wqkv, Wo):
    from concourse import bass_utils

    # zero-egress container: artifact upload is impossible and only feeds
    # trace metadata — replace with a local marker.
    bass_utils.upload_artifacts = lambda tmpdir: f"local://{tmpdir}"

    nc = _get_nc()
    in_maps = _prep_in_maps(x, Wqkv, Wo)
    trace = bool(os.environ.get("KERNEL_TRACE"))
    res = bass_utils.run_bass_kernel_spmd(
        nc, in_maps, core_ids=list(range(N_CORES)), trace=trace
    )
    LAST["exec_time_ns"] = res.exec_time_ns
    LAST["trace"] = res.instructions_and_trace
    acc = np.zeros((L, D), np.float32)
    for r in res.results:
        acc += r["y"].reshape(L, D).astype(np.float32)
    return acc.reshape(1, L, D).astype(np.float32)


# revision 9
# speedup vs baseline: 1.0281x; 1.0006x over previous
"""Multi-head attention (B=1, L=4096, D=512, H=8, DH=64) on 8 TRN2 NeuronCores.

Head-parallel: core h computes head h end-to-end, host reduces partial
y contributions (y_h = attn_h @ Wo[h*64:(h+1)*64, :]).

v4 — PE stream reduction + split exp + fine-grained post scheduling:
  - kq projection is ONE psum pass per i-tile (lhsT = [wq|wk] slab, w
    cols 64:192 of the [wq|wq|wk|wk|wv] layout): psum rows 0:64 = qT,
    rows 64:128 = kT. The row-offset duplicates that score-pairing
    needs come from two SBUF->SBUF partition-shift DMAs on the idle
    GpSimd ring instead of a second 512-col PE pass (i-tile 0 keeps the
    two-pass form). i-tile-0 groups are emitted LAST in each prologue
    iteration so the swap DMAs have cover.
  - exp of each [128,1024] score group is split by column range across
    ScalarE (table exp, 0:544) and VectorE (Schraudolph tensor_scalar,
    544:1024) so the stp psum bank pair drains in ~0.7us instead of
    1.1-1.2us; the score matmul recycling those banks two groups later
    stops stalling. 544/480 balances the engines' ns/col.
  - posts are cut into 9 small pieces pumped one-per-group (g>=6, when
    pv(i-1) is provably complete) so no single pumped piece injects
    >0.8us into an engine queue ahead of a latency-critical exp half,
    and no queued semaphore wait can park an engine: in-order engine
    queues suffer priority inversion otherwise.
  - output projection row-group-paired for i-tiles 0..6 (wo at
    partitions 0:64 AND 64:128; outT partition-shifted by GpSimd DMA):
    2 concurrent-pair streams instead of 4.
  - v-proj accumulates all 4 t-chunks in ONE psum tile, drained by a
    single batched copy (saves 3 DVE instruction overheads/tile).
  - last i-tile's posts chunked per 128 queries and pipelined across
    engines; y partials written fp16 (host reduces in f32).
  - All matmuls fp16. (fp8 DoubleRow: 2x MACs but the dual-fp8
    ldweights 64-col cap forces a second denominator matmul that
    exactly cancels the gain; DoublePixel measured no faster.)
  - Score tiles stay transposed (ST[j,i] = k_j.q_i) so P@V contracts
    over j with PT as the moving operand; per-query exp sums come from
    the ones-column 64 of vext.
"""

import os

import numpy as np

import concourse.bass as bass
import concourse.mybir as mybir
import concourse.tile as tile
from concourse import bacc
from concourse.bass import ts

F32 = mybir.dt.float32
F16 = mybir.dt.float16
I16 = mybir.dt.int16

L = 4096  # sequence length
D = 512  # model dim
H = 8  # heads
DH = 64  # head dim
P = 128  # partitions
DC = D // P  # d-chunks for the projection contraction (4)
IW = 512  # i-tile (query) width
NI = L // IW  # 8
NJ = L // P  # 32 j-tiles (key blocks)
GJ = 2  # j-tiles per exp group (2 PSUM banks per instruction)
NG = NJ // GJ  # groups per i-tile (16)
WCOLS = 320  # q-dup(128) + k-dup(128) + v(64)
N_CORES = 8
SPLIT = 544  # scalar/vector exp column split inside a [128,1024] group

LOG2E = 1.4426950408889634
A16 = 1024.0 * LOG2E * 0.125
B16 = 15360.0 - 44.7257

_CACHE = {}
LAST = {}


def build_bass():
    nc = bacc.Bacc(
        "TRN2", target_bir_lowering=False, debug=False, num_devices=N_CORES
    )
    xt = nc.dram_tensor("xt", [DC, P, L], F16, kind="ExternalInput")
    w = nc.dram_tensor("w", [DC, P, WCOLS], F16, kind="ExternalInput")
    wo = nc.dram_tensor("wo", [DH, D], F16, kind="ExternalInput")
    y = nc.dram_tensor("y", [L // P, P, D], F16, kind="ExternalOutput")

    with (
        tile.TileContext(nc) as tc,
        tc.tile_pool(name="const", bufs=1) as cpool,
        tc.tile_pool(name="ps", bufs=1, space="PSUM") as ppool,
        tc.tile_pool(name="pt", bufs=1) as pt_pool,
        tc.tile_pool(name="post", bufs=1) as post_pool,
        tc.tile_pool(name="yout", bufs=1) as yout_pool,
    ):
        x_sb = cpool.tile([P, DC, L], F16)
        w_sb = cpool.tile([P, DC, WCOLS], F16)
        wo2 = cpool.tile([P, D], F16)
        # first-needed first: w chunks and i-tile-0 x chunks interleaved
        # across the three DMA dispatch rings
        nc.sync.dma_start(w_sb[:, 0], w[0])
        nc.scalar.dma_start(w_sb[:, 1], w[1])
        nc.gpsimd.dma_start(w_sb[:, 2], w[2])
        nc.sync.dma_start(w_sb[:, 3], w[3])
        for c, e in zip(range(DC), (nc.gpsimd, nc.scalar, nc.gpsimd, nc.sync)):
            e.dma_start(x_sb[:, c, ts(0, IW)], xt[c, :, ts(0, IW)])
        nc.scalar.dma_start(wo2[0:DH], wo[:])
        nc.scalar.dma_start(wo2[DH:P], wo[:])
        for i in range(1, NI):
            nc.sync.dma_start(
                x_sb[:, :, ts(i, IW)],
                xt[:, :, ts(i, IW)].rearrange("c p l -> p c l"),
            )

        qdup = cpool.tile([P, L], F16)  # qT in rows 0:64 AND 64:128
        kdup = cpool.tile([P, L], F16)
        vext = cpool.tile([P, NJ, DH + 2], F16)
        nc.vector.memset(vext[:, :, DH], 1.0)
        # warm the ACT exp table while DMAs run
        warm = cpool.tile([1, 8], F32)
        nc.vector.memset(warm[:], 0.0)
        nc.scalar.activation(warm[:], warm[:], mybir.ActivationFunctionType.Exp)

        def emit_proj_kq(i2):
            if i2 == 0:
                # two full passes: no swap-DMA latency ahead of the very
                # first score groups. k first (it gates the j-tiles).
                for off, dst in ((P, kdup), (0, qdup)):
                    ps = ppool.tile([P, IW], F32, tag="proj", bufs=2, name="ps")
                    for c in range(DC):
                        nc.tensor.matmul(
                            ps[:],
                            lhsT=w_sb[:, c, off : off + P],
                            rhs=x_sb[:, c, ts(0, IW)],
                            start=(c == 0),
                            stop=(c == DC - 1),
                        )
                    nc.scalar.copy(dst[:, ts(0, IW)], ps[:])
                return
            # single pass: lhsT = [wq|wk] (w cols 64:192) -> psum rows
            # 0:64 = qT, rows 64:128 = kT; row-offset dups via DMA
            ps = ppool.tile([P, IW], F32, tag="proj", bufs=2, name="ps")
            for c in range(DC):
                nc.tensor.matmul(
                    ps[:],
                    lhsT=w_sb[:, c, DH : DH + P],
                    rhs=x_sb[:, c, ts(i2, IW)],
                    start=(c == 0),
                    stop=(c == DC - 1),
                )
            nc.scalar.copy(qdup[0:DH, ts(i2, IW)], ps[0:DH, :])
            nc.vector.tensor_copy(kdup[DH:P, ts(i2, IW)], ps[DH:P, :])
            nc.gpsimd.dma_start(qdup[DH:P, ts(i2, IW)], qdup[0:DH, ts(i2, IW)])
            nc.gpsimd.dma_start(kdup[0:DH, ts(i2, IW)], kdup[DH:P, ts(i2, IW)])

        def emit_proj_v(i2):
            # v in row layout: all 4 t-chunks accumulate in ONE psum tile,
            # drained by a single batched copy
            psv = ppool.tile([P, 4, DH], F32, tag="proj", bufs=2, name="psv")
            for t in range(4):
                for c in range(DC):
                    nc.tensor.matmul(
                        psv[:, t],
                        lhsT=x_sb[:, c, ts(4 * i2 + t, P)],
                        rhs=w_sb[:, c, 2 * P : 2 * P + DH],
                        start=(c == 0),
                        stop=(c == DC - 1),
                        skip_group_check=True,
                    )
            nc.vector.tensor_copy(vext[:, 4 * i2 : 4 * i2 + 4, 0:DH], psv[:])

        pvs = {}
        outTs = {}
        outT2s = {}
        # PV matmuls lag the score/exp emission by PV_LAG groups so the
        # in-order PE stream never parks on a PV that is waiting for its
        # exp: scores of the next groups issue first.
        PV_LAG = 4
        pv_q = []

        def flush_pv(limit):
            while len(pv_q) > limit:
                i, g, pt = pv_q.pop(0)
                for u in range(GJ):
                    jt = g * GJ + u
                    nc.tensor.matmul(
                        pvs[i][:],
                        lhsT=vext[:, jt, 0 : DH + 1],
                        rhs=pt[:, ts(u, IW)],
                        start=(jt == 0),
                        stop=(jt == NJ - 1),
                        skip_group_check=True,
                    )

        def emit_group(i, g, lag=PV_LAG):
            if g == 0:
                pvs[i] = ppool.tile(
                    [DH + 1, IW], F32, tag="acc", bufs=2, name=f"pv{i}"
                )
            stp = ppool.tile([P, GJ * IW], F32, tag="st", bufs=2, name="stp")
            for u in range(GJ):
                jt = g * GJ + u
                half = DH * (jt % 2)
                nc.tensor.matmul(
                    stp[:, ts(u, IW)],
                    lhsT=kdup[half : half + DH, ts(jt, P)],
                    rhs=qdup[half : half + DH, ts(i, IW)],
                    start=True,
                    stop=True,
                )
            pt = pt_pool.tile([P, GJ * IW], F16, tag="pt", bufs=20, name="pt")
            nc.scalar.activation(
                pt[:, 0:SPLIT],
                stp[:, 0:SPLIT],
                mybir.ActivationFunctionType.Exp,
                scale=0.125,
            )
            nc.vector.tensor_scalar(
                pt[:, SPLIT : GJ * IW].bitcast(I16),
                stp[:, SPLIT : GJ * IW],
                A16,
                B16,
                mybir.AluOpType.mult,
                mybir.AluOpType.add,
            )
            pv_q.append((i, g, pt))
            flush_pv(lag)

        # --- posts, cut into small per-engine pieces ---
        rcps = {}
        rbs = {}

        def post_rcp(i):
            pv = pvs[i]
            srow = post_pool.tile([1, IW], F32, tag="srow", bufs=2, name="srow")
            nc.scalar.copy(srow[:], pv[DH : DH + 1, :])
            rcp = post_pool.tile([1, IW], F32, tag="rcp1", bufs=2, name="rcp")
            nc.vector.reciprocal_approx_fast(rcp[:], srow[:])
            rcps[i] = rcp

        def post_bcast(i):
            rb = post_pool.tile([DH, IW], F32, tag="rb", bufs=2, name="rb")
            nc.gpsimd.partition_broadcast(rb[:], rcps[i][:])
            rbs[i] = rb

        def post_mul(i, h):
            if h == 0:
                outTs[i] = post_pool.tile(
                    [DH, IW], F16, tag="outT", bufs=2, name="outT"
                )
            sl = slice(h * (IW // 2), (h + 1) * (IW // 2))
            nc.vector.tensor_mul(
                outTs[i][:, sl], pvs[i][0:DH, sl], rbs[i][:, sl]
            )

        def post_outT2(i):
            # partition-shifted copy of outT for out-proj row-group pairing
            o2 = post_pool.tile([P, IW], F16, tag="outT2", bufs=2, name="outT2")
            nc.gpsimd.dma_start(o2[DH:P, :], outTs[i][:, :])
            outT2s[i] = o2

        def post_ymm(i, p):
            # paired out-proj: chunk 2p on PE rows 0:64, chunk 2p+1 on 64:128
            ya = ppool.tile([P, D], F32, tag="proj", bufs=2, name="ya")
            yb = ppool.tile([P, D], F32, tag="proj", bufs=2, name="yb")
            nc.tensor.matmul(
                ya[:],
                lhsT=outTs[i][:, ts(2 * p, P)],
                rhs=wo2[0:DH],
                start=True,
                stop=True,
            )
            nc.tensor.matmul(
                yb[:],
                lhsT=outT2s[i][DH:P, ts(2 * p + 1, P)],
                rhs=wo2[DH:P],
                start=True,
                stop=True,
            )
            ysa = yout_pool.tile([P, D], F16, tag="ysb", bufs=4, name="ysa")
            nc.scalar.copy(ysa[:], ya[:])
            nc.sync.dma_start(y[i * (IW // P) + 2 * p], ysa[:])
            return yb

        def post_yb(i, p, yb):
            ysb_ = yout_pool.tile([P, D], F16, tag="ysb", bufs=4, name="ysb")
            nc.vector.tensor_copy(ysb_[:], yb[:])
            nc.sync.dma_start(y[i * (IW // P) + 2 * p + 1], ysb_[:])

        def pend_posts(pending, i):
            ybs = {}

            def mk_ymm(p):
                def f():
                    ybs[p] = post_ymm(i, p)

                return f

            pending.append(lambda: post_rcp(i))
            pending.append(lambda: post_bcast(i))
            pending.append(lambda: post_mul(i, 0))
            pending.append(lambda: post_mul(i, 1))
            pending.append(lambda: post_outT2(i))
            pending.append(mk_ymm(0))
            pending.append(lambda: post_yb(i, 0, ybs[0]))
            pending.append(mk_ymm(1))
            pending.append(lambda: post_yb(i, 1, ybs[1]))

        def emit_post_tail(i):
            # last i-tile: phase-ordered emission (all srows, all rcps, ...)
            # so each in-order engine queue holds every chunk's work for
            # phase k before phase k+1 — per-chunk interleaved emission
            # serializes chunks through the queues instead.
            pv = pvs[i]
            NT = IW // P
            outT = post_pool.tile([DH, IW], F16, tag="outT", bufs=2, name="outTt")
            srows, rcpc, rbc = [], [], []
            for t in range(NT):
                srow = post_pool.tile([1, P], F32, tag="srowc", bufs=4, name="srowc")
                nc.scalar.copy(srow[:], pv[DH : DH + 1, ts(t, P)])
                srows.append(srow)
            for t in range(NT):
                rcp = post_pool.tile([1, P], F32, tag="rcpc", bufs=4, name="rcpc")
                nc.vector.reciprocal_approx_fast(rcp[:], srows[t][:])
                rcpc.append(rcp)
            for t in range(NT):
                rb = post_pool.tile([DH, P], F32, tag="rbc", bufs=4, name="rbc")
                nc.gpsimd.partition_broadcast(rb[:], rcpc[t][:])
                rbc.append(rb)
            for t in range(NT):
                nc.vector.tensor_mul(
                    outT[:, ts(t, P)], pv[0:DH, ts(t, P)], rbc[t][:]
                )
            for t in range(NT):
                # score psum banks are free now so yps alternates tags
                yps = ppool.tile(
                    [P, D],
                    F32,
                    tag="proj" if t % 2 == 0 else "st",
                    bufs=2,
                    name="yps",
                )
                nc.tensor.matmul(
                    yps[:],
                    lhsT=outT[:, ts(t, P)],
                    rhs=wo2[0:DH],
                    start=True,
                    stop=True,
                )
                ysb_ = yout_pool.tile([P, D], F16, tag="ysb", bufs=4, name="ysbt")
                if t % 2 == 0:
                    nc.scalar.copy(ysb_[:], yps[:])
                else:
                    nc.vector.tensor_copy(ysb_[:], yps[:])
                nc.sync.dma_start(y[i * (IW // P) + t], ysb_[:])

        # --- prologue: projections interleaved with i-tiles 0..2.
        # i-tile-0 groups go LAST in each iteration so the kq swap DMAs
        # for proj(i2) have the older tiles' groups as cover. ---
        from collections import deque

        pending = deque()

        def pump():
            if pending:
                pending.popleft()()

        for i2 in range(NI):
            emit_proj_kq(i2)
            emit_proj_v(i2)
            if i2 > 0:
                emit_group(1, 2 * (i2 - 1))
                emit_group(1, 2 * (i2 - 1) + 1)
            if i2 > 1:
                emit_group(2, 2 * (i2 - 2))
                emit_group(2, 2 * (i2 - 2) + 1)
            emit_group(0, 2 * i2)
            emit_group(0, 2 * i2 + 1)
        # trailing groups; the pv backlog is NOT bulk-flushed here — the
        # steady tiles run with a deeper pv lag (8) so the backlog drains
        # interleaved with tile-3's scores instead of as a monolithic
        # Tensor-queue drain that everything pumped afterward waits behind
        emit_group(1, NG - 2)
        emit_group(1, NG - 1)
        for g in range(2 * (NI - 2), NG):
            emit_group(2, g)
        for i in (0, 1, 2):
            pend_posts(pending, i)
        # --- steady state ---
        # Posts for earlier tiles are pumped only once the semaphore their
        # first piece waits on (that tile's pv complete) is satisfied
        # before the piece enters an engine queue: a queued wait parks the
        # whole in-order queue, including later exp halves. The deeper lag
        # (8) also means pv(i,0) — which needs the acc psum slot freed by
        # mul(i-2) — is emitted only at g==8, past that mul's pump slot.
        for i in range(3, NI):
            for g in range(NG):
                emit_group(i, g, lag=8)
                # tile 3 drains the three-tile backlog left by the
                # prologue (27 pieces over 28 slots), two per group
                if i == 3:
                    if g >= 2:
                        pump()
                        pump()
                elif g >= 7:
                    # g >= 7 only: pv(i-1, 15) is flushed inside
                    # emit_group(i, 7) — a post piece emitted before it
                    # would read partial accumulations
                    pump()
            if i < NI - 1:
                pend_posts(pending, i)
        flush_pv(0)
        while pending:
            pump()
        emit_post_tail(NI - 1)
    nc.compile()
    return nc


def _get_nc():
    if "nc" not in _CACHE:
        _CACHE["nc"] = build_bass()
    return _CACHE["nc"]


def _prep_in_maps(x, Wqkv, Wo):
    x = np.asarray(x, dtype=np.float32).reshape(L, D)
    Wqkv = np.asarray(Wqkv, dtype=np.float32)
    Wo = np.asarray(Wo, dtype=np.float32)
    xt = np.ascontiguousarray(x.T).reshape(DC, P, L).astype(np.float16)
    in_maps = []
    for h in range(N_CORES):
        wq = Wqkv[:, 0 * D + h * DH : 0 * D + (h + 1) * DH]
        wk = Wqkv[:, 1 * D + h * DH : 1 * D + (h + 1) * DH]
        wv = Wqkv[:, 2 * D + h * DH : 2 * D + (h + 1) * DH]
        cols = np.concatenate([wq, wq, wk, wk, wv], axis=1)  # [512, 320]
        w_dram = np.ascontiguousarray(cols).reshape(DC, P, WCOLS).astype(np.float16)
        wo_h = np.ascontiguousarray(Wo[h * DH : (h + 1) * DH, :]).astype(np.float16)
        in_maps.append({"xt": xt, "w": w_dram, "wo": wo_h})
    return in_maps


def kernel(x, Wqkv, Wo):
    from concourse import bass_utils

    # zero-egress container: artifact upload is impossible and only feeds
    # trace metadata — replace with a local marker.
    bass_utils.upload_artifacts = lambda tmpdir: f"local://{tmpdir}"

    nc = _get_nc()
    in_maps = _prep_in_maps(x, Wqkv, Wo)
    trace = bool(os.environ.get("KERNEL_TRACE"))
    res = bass_utils.run_bass_kernel_spmd(
        nc, in_maps, core_ids=list(range(N_CORES)), trace=trace
    )
    LAST["exec_time_ns"] = res.exec_time_ns
    LAST["trace"] = res.instructions_and_trace
    acc = np.zeros((L, D), np.float32)
    for r in res.results:
        acc += r["y"].reshape(L, D).astype(np.float32)
    return acc.reshape(1, L, D).astype(np.float32)


# revision 11
# speedup vs baseline: 1.1507x; 1.1193x over previous
"""Multi-head attention (B=1, L=4096, D=512, H=8, DH=64) on 8 TRN2 NeuronCores.

Head-parallel: core h computes head h end-to-end, host reduces partial
y contributions (y_h = attn_h @ Wo[h*64:(h+1)*64, :]).

v4 — PE stream reduction + split exp + fine-grained post scheduling:
  - kq projection is ONE psum pass per i-tile (lhsT = [wq|wk] slab, w
    cols 64:192 of the [wq|wq|wk|wk|wv] layout): psum rows 0:64 = qT,
    rows 64:128 = kT. The row-offset duplicates that score-pairing
    needs come from two SBUF->SBUF partition-shift DMAs on the idle
    GpSimd ring instead of a second 512-col PE pass (i-tile 0 keeps the
    two-pass form). i-tile-0 groups are emitted LAST in each prologue
    iteration so the swap DMAs have cover.
  - exp of each [128,1024] score group is split by column range across
    ScalarE (table exp, 0:544) and VectorE (Schraudolph tensor_scalar,
    544:1024) so the stp psum bank pair drains in ~0.7us instead of
    1.1-1.2us; the score matmul recycling those banks two groups later
    stops stalling. 544/480 balances the engines' ns/col.
  - posts are cut into 9 small pieces pumped one-per-group (g>=6, when
    pv(i-1) is provably complete) so no single pumped piece injects
    >0.8us into an engine queue ahead of a latency-critical exp half,
    and no queued semaphore wait can park an engine: in-order engine
    queues suffer priority inversion otherwise.
  - output projection row-group-paired for i-tiles 0..6 (wo at
    partitions 0:64 AND 64:128; outT partition-shifted by GpSimd DMA):
    2 concurrent-pair streams instead of 4.
  - v-proj accumulates all 4 t-chunks in ONE psum tile, drained by a
    single batched copy (saves 3 DVE instruction overheads/tile).
  - last i-tile's posts chunked per 128 queries and pipelined across
    engines; y partials written fp16 (host reduces in f32).
  - All matmuls fp16. (fp8 DoubleRow: 2x MACs but the dual-fp8
    ldweights 64-col cap forces a second denominator matmul that
    exactly cancels the gain; DoublePixel measured no faster.)
  - Score tiles stay transposed (ST[j,i] = k_j.q_i) so P@V contracts
    over j with PT as the moving operand; per-query exp sums come from
    the ones-column 64 of vext.
"""

import os

import numpy as np

import concourse.bass as bass
import concourse.mybir as mybir
import concourse.tile as tile
from concourse import bacc
from concourse.bass import ts

F32 = mybir.dt.float32
F16 = mybir.dt.float16
I16 = mybir.dt.int16

L = 4096  # sequence length
D = 512  # model dim
H = 8  # heads
DH = 64  # head dim
P = 128  # partitions
DC = D // P  # d-chunks for the projection contraction (4)
IW = 512  # i-tile (query) width
NI = L // IW  # 8
NJ = L // P  # 32 j-tiles (key blocks)
GJ = 2  # j-tiles per exp group (2 PSUM banks per instruction)
NG = NJ // GJ  # groups per i-tile (16)
WCOLS = 320  # q-dup(128) + k-dup(128) + v(64)
N_CORES = 8

# per-i-tile engine pattern for the 16 exp groups: A=ScalarE table exp,
# D=VectorE Schraudolph. Whole-group alternation costs less total engine
# time than column-splitting every group across both engines (each extra
# instruction pays ~80-190ns fixed overhead).
PATTERN = ["A", "A", "D", "A", "D", "A", "D", "A",
           "D", "A", "D", "A", "D", "A", "D", "A"]

LOG2E = 1.4426950408889634
A16 = 1024.0 * LOG2E * 0.125
B16 = 15360.0 - 44.7257

_CACHE = {}
LAST = {}


def build_bass():
    nc = bacc.Bacc(
        "TRN2", target_bir_lowering=False, debug=False, num_devices=N_CORES
    )
    xt = nc.dram_tensor("xt", [DC, P, L], F16, kind="ExternalInput")
    w = nc.dram_tensor("w", [DC, P, WCOLS], F16, kind="ExternalInput")
    wo = nc.dram_tensor("wo", [DH, D], F16, kind="ExternalInput")
    y = nc.dram_tensor("y", [L // P, P, D], F16, kind="ExternalOutput")

    with (
        tile.TileContext(nc) as tc,
        tc.tile_pool(name="const", bufs=1) as cpool,
        tc.tile_pool(name="ps", bufs=1, space="PSUM") as ppool,
        tc.tile_pool(name="pt", bufs=1) as pt_pool,
        tc.tile_pool(name="post", bufs=1) as post_pool,
        tc.tile_pool(name="yout", bufs=1) as yout_pool,
    ):
        x_sb = cpool.tile([P, DC, L], F16)
        w_sb = cpool.tile([P, DC, WCOLS], F16)
        wo2 = cpool.tile([P, D], F16)
        # first-needed first: w chunks and i-tile-0 x chunks interleaved
        # across the three DMA dispatch rings
        nc.sync.dma_start(w_sb[:, 0], w[0])
        nc.scalar.dma_start(w_sb[:, 1], w[1])
        nc.gpsimd.dma_start(w_sb[:, 2], w[2])
        nc.sync.dma_start(w_sb[:, 3], w[3])
        for c, e in zip(range(DC), (nc.gpsimd, nc.scalar, nc.gpsimd, nc.sync)):
            e.dma_start(x_sb[:, c, ts(0, IW)], xt[c, :, ts(0, IW)])
        nc.scalar.dma_start(wo2[0:DH], wo[:])
        nc.scalar.dma_start(wo2[DH:P], wo[:])
        for i in range(1, NI):
            nc.sync.dma_start(
                x_sb[:, :, ts(i, IW)],
                xt[:, :, ts(i, IW)].rearrange("c p l -> p c l"),
            )

        qdup = cpool.tile([P, L], F16)  # qT in rows 0:64 AND 64:128
        kdup = cpool.tile([P, L], F16)
        vext = cpool.tile([P, NJ, DH + 2], F16)
        nc.vector.memset(vext[:, :, DH], 1.0)
        # warm the ACT exp table while DMAs run
        warm = cpool.tile([1, 8], F32)
        nc.vector.memset(warm[:], 0.0)
        nc.scalar.activation(warm[:], warm[:], mybir.ActivationFunctionType.Exp)

        def emit_proj_kq(i2):
            if i2 == 0:
                # two full passes: no swap-DMA latency ahead of the very
                # first score groups. k first (it gates the j-tiles).
                for off, dst in ((P, kdup), (0, qdup)):
                    ps = ppool.tile([P, IW], F32, tag="proj", bufs=2, name="ps")
                    for c in range(DC):
                        nc.tensor.matmul(
                            ps[:],
                            lhsT=w_sb[:, c, off : off + P],
                            rhs=x_sb[:, c, ts(0, IW)],
                            start=(c == 0),
                            stop=(c == DC - 1),
                        )
                    nc.scalar.copy(dst[:, ts(0, IW)], ps[:])
                return
            # single pass: lhsT = [wq|wk] (w cols 64:192) -> psum rows
            # 0:64 = qT, rows 64:128 = kT; row-offset dups via DMA
            ps = ppool.tile([P, IW], F32, tag="proj", bufs=2, name="ps")
            for c in range(DC):
                nc.tensor.matmul(
                    ps[:],
                    lhsT=w_sb[:, c, DH : DH + P],
                    rhs=x_sb[:, c, ts(i2, IW)],
                    start=(c == 0),
                    stop=(c == DC - 1),
                )
            nc.scalar.copy(qdup[0:DH, ts(i2, IW)], ps[0:DH, :])
            nc.vector.tensor_copy(kdup[DH:P, ts(i2, IW)], ps[DH:P, :])
            nc.gpsimd.dma_start(qdup[DH:P, ts(i2, IW)], qdup[0:DH, ts(i2, IW)])
            nc.gpsimd.dma_start(kdup[0:DH, ts(i2, IW)], kdup[DH:P, ts(i2, IW)])

        def emit_proj_v(i2):
            # v in row layout: all 4 t-chunks accumulate in ONE psum tile,
            # drained by a single batched copy
            psv = ppool.tile([P, 4, DH], F32, tag="proj", bufs=2, name="psv")
            for t in range(4):
                for c in range(DC):
                    nc.tensor.matmul(
                        psv[:, t],
                        lhsT=x_sb[:, c, ts(4 * i2 + t, P)],
                        rhs=w_sb[:, c, 2 * P : 2 * P + DH],
                        start=(c == 0),
                        stop=(c == DC - 1),
                        skip_group_check=True,
                    )
            nc.vector.tensor_copy(vext[:, 4 * i2 : 4 * i2 + 4, 0:DH], psv[:])

        pvs = {}
        outTs = {}
        outT2s = {}
        # PV matmuls lag the score/exp emission by PV_LAG groups so the
        # in-order PE stream never parks on a PV that is waiting for its
        # exp: scores of the next groups issue first.
        PV_LAG = 4
        pv_q = []

        def flush_pv(limit):
            while len(pv_q) > limit:
                i, g, pt = pv_q.pop(0)
                for u in range(GJ):
                    jt = g * GJ + u
                    nc.tensor.matmul(
                        pvs[i][:],
                        lhsT=vext[:, jt, 0 : DH + 1],
                        rhs=pt[:, ts(u, IW)],
                        start=(jt == 0),
                        stop=(jt == NJ - 1),
                        skip_group_check=True,
                    )

        def emit_group(i, g, lag=PV_LAG):
            if g == 0:
                pvs[i] = ppool.tile(
                    [DH + 1, IW], F32, tag="acc", bufs=2, name=f"pv{i}"
                )
            stp = ppool.tile([P, GJ * IW], F32, tag="st", bufs=2, name="stp")
            for u in range(GJ):
                jt = g * GJ + u
                half = DH * (jt % 2)
                nc.tensor.matmul(
                    stp[:, ts(u, IW)],
                    lhsT=kdup[half : half + DH, ts(jt, P)],
                    rhs=qdup[half : half + DH, ts(i, IW)],
                    start=True,
                    stop=True,
                )
            pt = pt_pool.tile([P, GJ * IW], F16, tag="pt", bufs=20, name="pt")
            if PATTERN[g] == "A":
                nc.scalar.activation(
                    pt[:],
                    stp[:],
                    mybir.ActivationFunctionType.Exp,
                    scale=0.125,
                )
            else:
                nc.vector.tensor_scalar(
                    pt[:].bitcast(I16),
                    stp[:],
                    A16,
                    B16,
                    mybir.AluOpType.mult,
                    mybir.AluOpType.add,
                )
            pv_q.append((i, g, pt))
            flush_pv(lag)

        # --- posts, cut into small per-engine pieces ---
        rcps = {}
        rbs = {}

        def post_rcp(i):
            pv = pvs[i]
            srow = post_pool.tile([1, IW], F32, tag="srow", bufs=2, name="srow")
            nc.scalar.copy(srow[:], pv[DH : DH + 1, :])
            rcp = post_pool.tile([1, IW], F32, tag="rcp1", bufs=2, name="rcp")
            nc.vector.reciprocal_approx_fast(rcp[:], srow[:])
            rcps[i] = rcp

        def post_bcast(i):
            rb = post_pool.tile([DH, IW], F32, tag="rb", bufs=2, name="rb")
            nc.gpsimd.partition_broadcast(rb[:], rcps[i][:])
            rbs[i] = rb

        def post_mul(i, h):
            if h == 0:
                outTs[i] = post_pool.tile(
                    [DH, IW], F16, tag="outT", bufs=2, name="outT"
                )
            sl = slice(h * (IW // 2), (h + 1) * (IW // 2))
            nc.vector.tensor_mul(
                outTs[i][:, sl], pvs[i][0:DH, sl], rbs[i][:, sl]
            )

        def post_outT2(i):
            # partition-shifted copy of outT for out-proj row-group pairing
            o2 = post_pool.tile([P, IW], F16, tag="outT2", bufs=2, name="outT2")
            nc.gpsimd.dma_start(o2[DH:P, :], outTs[i][:, :])
            outT2s[i] = o2

        def post_ymm(i, p):
            # paired out-proj: chunk 2p on PE rows 0:64, chunk 2p+1 on 64:128
            ya = ppool.tile([P, D], F32, tag="proj", bufs=2, name="ya")
            yb = ppool.tile([P, D], F32, tag="proj", bufs=2, name="yb")
            nc.tensor.matmul(
                ya[:],
                lhsT=outTs[i][:, ts(2 * p, P)],
                rhs=wo2[0:DH],
                start=True,
                stop=True,
            )
            nc.tensor.matmul(
                yb[:],
                lhsT=outT2s[i][DH:P, ts(2 * p + 1, P)],
                rhs=wo2[DH:P],
                start=True,
                stop=True,
            )
            ysa = yout_pool.tile([P, D], F16, tag="ysb", bufs=4, name="ysa")
            nc.scalar.copy(ysa[:], ya[:])
            nc.sync.dma_start(y[i * (IW // P) + 2 * p], ysa[:])
            return yb

        def post_yb(i, p, yb):
            ysb_ = yout_pool.tile([P, D], F16, tag="ysb", bufs=4, name="ysb")
            nc.vector.tensor_copy(ysb_[:], yb[:])
            nc.sync.dma_start(y[i * (IW // P) + 2 * p + 1], ysb_[:])

        def pend_posts(pending, i):
            ybs = {}

            def mk_ymm(p):
                def f():
                    ybs[p] = post_ymm(i, p)

                return f

            pending.append(lambda: post_rcp(i))
            pending.append(lambda: post_bcast(i))
            pending.append(lambda: post_mul(i, 0))
            pending.append(lambda: post_mul(i, 1))
            pending.append(lambda: post_outT2(i))
            pending.append(mk_ymm(0))
            pending.append(lambda: post_yb(i, 0, ybs[0]))
            pending.append(mk_ymm(1))
            pending.append(lambda: post_yb(i, 1, ybs[1]))

        def emit_post_tail(i):
            # last i-tile: phase-ordered emission (all srows, all rcps, ...)
            # so each in-order engine queue holds every chunk's work for
            # phase k before phase k+1 — per-chunk interleaved emission
            # serializes chunks through the queues instead.
            pv = pvs[i]
            NT = IW // P
            outT = post_pool.tile([DH, IW], F16, tag="outT", bufs=2, name="outTt")
            srows, rcpc, rbc = [], [], []
            for t in range(NT):
                srow = post_pool.tile([1, P], F32, tag="srowc", bufs=4, name="srowc")
                nc.scalar.copy(srow[:], pv[DH : DH + 1, ts(t, P)])
                srows.append(srow)
            for t in range(NT):
                rcp = post_pool.tile([1, P], F32, tag="rcpc", bufs=4, name="rcpc")
                nc.vector.reciprocal_approx_fast(rcp[:], srows[t][:])
                rcpc.append(rcp)
            for t in range(NT):
                rb = post_pool.tile([DH, P], F32, tag="rbc", bufs=4, name="rbc")
                nc.gpsimd.partition_broadcast(rb[:], rcpc[t][:])
                rbc.append(rb)
            for t in range(NT):
                nc.vector.tensor_mul(
                    outT[:, ts(t, P)], pv[0:DH, ts(t, P)], rbc[t][:]
                )
            for t in range(NT):
                # score psum banks are free now so yps alternates tags
                yps = ppool.tile(
                    [P, D],
                    F32,
                    tag="proj" if t % 2 == 0 else "st",
                    bufs=2,
                    name="yps",
                )
                nc.tensor.matmul(
                    yps[:],
                    lhsT=outT[:, ts(t, P)],
                    rhs=wo2[0:DH],
                    start=True,
                    stop=True,
                )
                ysb_ = yout_pool.tile([P, D], F16, tag="ysb", bufs=4, name="ysbt")
                if t % 2 == 0:
                    nc.scalar.copy(ysb_[:], yps[:])
                else:
                    nc.vector.tensor_copy(ysb_[:], yps[:])
                nc.sync.dma_start(y[i * (IW // P) + t], ysb_[:])

        # --- prologue: projections interleaved with i-tiles 0..2.
        # i-tile-0 groups go LAST in each iteration so the kq swap DMAs
        # for proj(i2) have the older tiles' groups as cover. ---
        from collections import deque

        pending = deque()

        def pump():
            if pending:
                pending.popleft()()

        for i2 in range(NI):
            emit_proj_kq(i2)
            emit_proj_v(i2)
            if i2 > 0:
                emit_group(1, 2 * (i2 - 1))
                emit_group(1, 2 * (i2 - 1) + 1)
            if i2 > 1:
                emit_group(2, 2 * (i2 - 2))
                emit_group(2, 2 * (i2 - 2) + 1)
            emit_group(0, 2 * i2)
            emit_group(0, 2 * i2 + 1)
        # trailing groups; the pv backlog is NOT bulk-flushed here — the
        # steady tiles run with a deeper pv lag (8) so the backlog drains
        # interleaved with tile-3's scores instead of as a monolithic
        # Tensor-queue drain that everything pumped afterward waits behind
        emit_group(1, NG - 2)
        emit_group(1, NG - 1)
        for g in range(2 * (NI - 2), NG):
            emit_group(2, g)
        for i in (0, 1, 2):
            pend_posts(pending, i)
        # --- steady state ---
        # Posts for earlier tiles are pumped only once the semaphore their
        # first piece waits on (that tile's pv complete) is satisfied
        # before the piece enters an engine queue: a queued wait parks the
        # whole in-order queue, including later exp halves. The deeper lag
        # (8) also means pv(i,0) — which needs the acc psum slot freed by
        # mul(i-2) — is emitted only at g==8, past that mul's pump slot.
        for i in range(3, NI):
            for g in range(NG):
                emit_group(i, g, lag=8)
                # tile 3 drains the three-tile backlog left by the
                # prologue (27 pieces over 28 slots), two per group
                if i == 3:
                    if g >= 2:
                        pump()
                        pump()
                elif g >= 7:
                    # g >= 7 only: pv(i-1, 15) is flushed inside
                    # emit_group(i, 7) — a post piece emitted before it
                    # would read partial accumulations
                    pump()
            if i < NI - 1:
                pend_posts(pending, i)
        flush_pv(0)
        while pending:
            pump()
        emit_post_tail(NI - 1)
    nc.compile()
    return nc


def _get_nc():
    if "nc" not in _CACHE:
        _CACHE["nc"] = build_bass()
    return _CACHE["nc"]


def _prep_in_maps(x, Wqkv, Wo):
    x = np.asarray(x, dtype=np.float32).reshape(L, D)
    Wqkv = np.asarray(Wqkv, dtype=np.float32)
    Wo = np.asarray(Wo, dtype=np.float32)
    xt = np.ascontiguousarray(x.T).reshape(DC, P, L).astype(np.float16)
    in_maps = []
    for h in range(N_CORES):
        wq = Wqkv[:, 0 * D + h * DH : 0 * D + (h + 1) * DH]
        wk = Wqkv[:, 1 * D + h * DH : 1 * D + (h + 1) * DH]
        wv = Wqkv[:, 2 * D + h * DH : 2 * D + (h + 1) * DH]
        cols = np.concatenate([wq, wq, wk, wk, wv], axis=1)  # [512, 320]
        w_dram = np.ascontiguousarray(cols).reshape(DC, P, WCOLS).astype(np.float16)
        wo_h = np.ascontiguousarray(Wo[h * DH : (h + 1) * DH, :]).astype(np.float16)
        in_maps.append({"xt": xt, "w": w_dram, "wo": wo_h})
    return in_maps


def kernel(x, Wqkv, Wo):
    from concourse import bass_utils

    # zero-egress container: artifact upload is impossible and only feeds
    # trace metadata — replace with a local marker.
    bass_utils.upload_artifacts = lambda tmpdir: f"local://{tmpdir}"

    nc = _get_nc()
    in_maps = _prep_in_maps(x, Wqkv, Wo)
    trace = bool(os.environ.get("KERNEL_TRACE"))
    res = bass_utils.run_bass_kernel_spmd(
        nc, in_maps, core_ids=list(range(N_CORES)), trace=trace
    )
    LAST["exec_time_ns"] = res.exec_time_ns
    LAST["trace"] = res.instructions_and_trace
    acc = np.zeros((L, D), np.float32)
    for r in res.results:
        acc += r["y"].reshape(L, D).astype(np.float32)
    return acc.reshape(1, L, D).astype(np.float32)


# revision 12
# speedup vs baseline: 1.1656x; 1.0129x over previous
"""Multi-head attention (B=1, L=4096, D=512, H=8, DH=64) on 8 TRN2 NeuronCores.

Head-parallel: core h computes head h end-to-end, host reduces partial
y contributions (y_h = attn_h @ Wo[h*64:(h+1)*64, :]).

v4 — PE stream reduction + split exp + fine-grained post scheduling:
  - kq projection is ONE psum pass per i-tile (lhsT = [wq|wk] slab, w
    cols 64:192 of the [wq|wq|wk|wk|wv] layout): psum rows 0:64 = qT,
    rows 64:128 = kT. The row-offset duplicates that score-pairing
    needs come from two SBUF->SBUF partition-shift DMAs on the idle
    GpSimd ring instead of a second 512-col PE pass (i-tile 0 keeps the
    two-pass form). i-tile-0 groups are emitted LAST in each prologue
    iteration so the swap DMAs have cover.
  - exp of each [128,1024] score group is split by column range across
    ScalarE (table exp, 0:544) and VectorE (Schraudolph tensor_scalar,
    544:1024) so the stp psum bank pair drains in ~0.7us instead of
    1.1-1.2us; the score matmul recycling those banks two groups later
    stops stalling. 544/480 balances the engines' ns/col.
  - posts are cut into 9 small pieces pumped one-per-group (g>=6, when
    pv(i-1) is provably complete) so no single pumped piece injects
    >0.8us into an engine queue ahead of a latency-critical exp half,
    and no queued semaphore wait can park an engine: in-order engine
    queues suffer priority inversion otherwise.
  - output projection row-group-paired for i-tiles 0..6 (wo at
    partitions 0:64 AND 64:128; outT partition-shifted by GpSimd DMA):
    2 concurrent-pair streams instead of 4.
  - v-proj accumulates all 4 t-chunks in ONE psum tile, drained by a
    single batched copy (saves 3 DVE instruction overheads/tile).
  - last i-tile's posts chunked per 128 queries and pipelined across
    engines; y partials written fp16 (host reduces in f32).
  - All matmuls fp16. (fp8 DoubleRow: 2x MACs but the dual-fp8
    ldweights 64-col cap forces a second denominator matmul that
    exactly cancels the gain; DoublePixel measured no faster.)
  - Score tiles stay transposed (ST[j,i] = k_j.q_i) so P@V contracts
    over j with PT as the moving operand; per-query exp sums come from
    the ones-column 64 of vext.
"""

import os

import numpy as np

import concourse.bass as bass
import concourse.mybir as mybir
import concourse.tile as tile
from concourse import bacc
from concourse.bass import ts

F32 = mybir.dt.float32
F16 = mybir.dt.float16
I16 = mybir.dt.int16

L = 4096  # sequence length
D = 512  # model dim
H = 8  # heads
DH = 64  # head dim
P = 128  # partitions
DC = D // P  # d-chunks for the projection contraction (4)
IW = 512  # i-tile (query) width
NI = L // IW  # 8
NJ = L // P  # 32 j-tiles (key blocks)
GJ = 2  # j-tiles per exp group (2 PSUM banks per instruction)
NG = NJ // GJ  # groups per i-tile (16)
WCOLS = 320  # q-dup(128) + k-dup(128) + v(64)
N_CORES = 8

# per-i-tile engine pattern for the 16 exp groups: A=ScalarE table exp,
# D=VectorE Schraudolph. Whole-group alternation costs less total engine
# time than column-splitting every group across both engines (each extra
# instruction pays ~80-190ns fixed overhead).
PATTERN = ["A", "A", "D", "A", "D", "A", "D", "A",
           "D", "A", "D", "A", "D", "A", "D", "A"]

LOG2E = 1.4426950408889634
A16 = 1024.0 * LOG2E * 0.125
B16 = 15360.0 - 44.7257

_CACHE = {}
LAST = {}


def build_bass():
    nc = bacc.Bacc(
        "TRN2", target_bir_lowering=False, debug=False, num_devices=N_CORES
    )
    xt = nc.dram_tensor("xt", [DC, P, L], F16, kind="ExternalInput")
    w = nc.dram_tensor("w", [DC, P, WCOLS], F16, kind="ExternalInput")
    wo = nc.dram_tensor("wo", [DH, D], F16, kind="ExternalInput")
    y = nc.dram_tensor("y", [L // P, P, D], F16, kind="ExternalOutput")

    with (
        tile.TileContext(nc) as tc,
        tc.tile_pool(name="const", bufs=1) as cpool,
        tc.tile_pool(name="ps", bufs=1, space="PSUM") as ppool,
        tc.tile_pool(name="pt", bufs=1) as pt_pool,
        tc.tile_pool(name="post", bufs=1) as post_pool,
        tc.tile_pool(name="yout", bufs=1) as yout_pool,
    ):
        x_sb = cpool.tile([P, DC, L], F16)
        w_sb = cpool.tile([P, DC, WCOLS], F16)
        wo2 = cpool.tile([P, D], F16)
        # first-needed first: w chunks and i-tile-0 x chunks interleaved
        # across the three DMA dispatch rings
        nc.sync.dma_start(w_sb[:, 0], w[0])
        nc.scalar.dma_start(w_sb[:, 1], w[1])
        nc.gpsimd.dma_start(w_sb[:, 2], w[2])
        nc.sync.dma_start(w_sb[:, 3], w[3])
        for c, e in zip(range(DC), (nc.gpsimd, nc.scalar, nc.gpsimd, nc.sync)):
            e.dma_start(x_sb[:, c, ts(0, IW)], xt[c, :, ts(0, IW)])
        nc.scalar.dma_start(wo2[0:DH], wo[:])
        nc.scalar.dma_start(wo2[DH:P], wo[:])
        for i in range(1, NI):
            nc.sync.dma_start(
                x_sb[:, :, ts(i, IW)],
                xt[:, :, ts(i, IW)].rearrange("c p l -> p c l"),
            )

        qdup = cpool.tile([P, L], F16)  # qT in rows 0:64 AND 64:128
        kdup = cpool.tile([P, L], F16)
        vext = cpool.tile([P, NJ, DH + 2], F16)
        nc.vector.memset(vext[:, :, DH], 1.0)
        # warm the ACT exp table while DMAs run
        warm = cpool.tile([1, 8], F32)
        nc.vector.memset(warm[:], 0.0)
        nc.scalar.activation(warm[:], warm[:], mybir.ActivationFunctionType.Exp)

        def emit_proj_kq(i2):
            if i2 == 0:
                # two full passes: no swap-DMA latency ahead of the very
                # first score groups. k first (it gates the j-tiles).
                for off, dst in ((P, kdup), (0, qdup)):
                    ps = ppool.tile([P, IW], F32, tag="proj", bufs=2, name="ps")
                    for c in range(DC):
                        nc.tensor.matmul(
                            ps[:],
                            lhsT=w_sb[:, c, off : off + P],
                            rhs=x_sb[:, c, ts(0, IW)],
                            start=(c == 0),
                            stop=(c == DC - 1),
                        )
                    nc.scalar.copy(dst[:, ts(0, IW)], ps[:])
                return
            # single pass: lhsT = [wq|wk] (w cols 64:192) -> psum rows
            # 0:64 = qT, rows 64:128 = kT; row-offset dups via DMA
            ps = ppool.tile([P, IW], F32, tag="proj", bufs=2, name="ps")
            for c in range(DC):
                nc.tensor.matmul(
                    ps[:],
                    lhsT=w_sb[:, c, DH : DH + P],
                    rhs=x_sb[:, c, ts(i2, IW)],
                    start=(c == 0),
                    stop=(c == DC - 1),
                )
            nc.scalar.copy(qdup[0:DH, ts(i2, IW)], ps[0:DH, :])
            nc.vector.tensor_copy(kdup[DH:P, ts(i2, IW)], ps[DH:P, :])
            nc.gpsimd.dma_start(qdup[DH:P, ts(i2, IW)], qdup[0:DH, ts(i2, IW)])
            nc.gpsimd.dma_start(kdup[0:DH, ts(i2, IW)], kdup[DH:P, ts(i2, IW)])

        def emit_proj_v(i2):
            # v in row layout: all 4 t-chunks accumulate in ONE psum tile,
            # drained by a single batched copy
            psv = ppool.tile([P, 4, DH], F32, tag="proj", bufs=2, name="psv")
            for t in range(4):
                for c in range(DC):
                    nc.tensor.matmul(
                        psv[:, t],
                        lhsT=x_sb[:, c, ts(4 * i2 + t, P)],
                        rhs=w_sb[:, c, 2 * P : 2 * P + DH],
                        start=(c == 0),
                        stop=(c == DC - 1),
                        skip_group_check=True,
                    )
            nc.vector.tensor_copy(vext[:, 4 * i2 : 4 * i2 + 4, 0:DH], psv[:])

        pvs = {}
        outTs = {}
        outT2s = {}
        # PV matmuls lag the score/exp emission by PV_LAG groups so the
        # in-order PE stream never parks on a PV that is waiting for its
        # exp: scores of the next groups issue first.
        PV_LAG = 4
        pv_q = []

        def flush_pv(limit):
            while len(pv_q) > limit:
                i, g, pt = pv_q.pop(0)
                for u in range(GJ):
                    jt = g * GJ + u
                    nc.tensor.matmul(
                        pvs[i][:],
                        lhsT=vext[:, jt, 0 : DH + 1],
                        rhs=pt[:, ts(u, IW)],
                        start=(jt == 0),
                        stop=(jt == NJ - 1),
                        skip_group_check=True,
                    )

        def emit_group(i, g, lag=PV_LAG):
            if g == 0:
                pvs[i] = ppool.tile(
                    [DH + 1, IW], F32, tag="acc", bufs=2, name=f"pv{i}"
                )
            stp = ppool.tile([P, GJ * IW], F32, tag="st", bufs=2, name="stp")
            for u in range(GJ):
                jt = g * GJ + u
                half = DH * (jt % 2)
                nc.tensor.matmul(
                    stp[:, ts(u, IW)],
                    lhsT=kdup[half : half + DH, ts(jt, P)],
                    rhs=qdup[half : half + DH, ts(i, IW)],
                    start=True,
                    stop=True,
                )
            pt = pt_pool.tile([P, GJ * IW], F16, tag="pt", bufs=20, name="pt")
            if PATTERN[g] == "A":
                nc.scalar.activation(
                    pt[:],
                    stp[:],
                    mybir.ActivationFunctionType.Exp,
                    scale=0.125,
                )
            else:
                nc.vector.tensor_scalar(
                    pt[:].bitcast(I16),
                    stp[:],
                    A16,
                    B16,
                    mybir.AluOpType.mult,
                    mybir.AluOpType.add,
                )
            pv_q.append((i, g, pt))
            # flush every other group: the Tensor queue then runs 4 score
            # matmuls followed by 4 pv matmuls, halving the pv<->score
            # weight-config transitions (~230ns pipeline bubble each)
            if g % 2 == 1:
                flush_pv(lag)

        # --- posts, cut into small per-engine pieces ---
        rcps = {}
        rbs = {}

        def post_rcp(i):
            pv = pvs[i]
            srow = post_pool.tile([1, IW], F32, tag="srow", bufs=2, name="srow")
            nc.scalar.copy(srow[:], pv[DH : DH + 1, :])
            rcp = post_pool.tile([1, IW], F32, tag="rcp1", bufs=2, name="rcp")
            nc.vector.reciprocal_approx_fast(rcp[:], srow[:])
            rcps[i] = rcp

        def post_bcast(i):
            rb = post_pool.tile([DH, IW], F32, tag="rb", bufs=2, name="rb")
            nc.gpsimd.partition_broadcast(rb[:], rcps[i][:])
            rbs[i] = rb

        def post_mul(i, h):
            if h == 0:
                outTs[i] = post_pool.tile(
                    [DH, IW], F16, tag="outT", bufs=2, name="outT"
                )
            sl = slice(h * (IW // 2), (h + 1) * (IW // 2))
            nc.vector.tensor_mul(
                outTs[i][:, sl], pvs[i][0:DH, sl], rbs[i][:, sl]
            )

        def post_outT2(i):
            # partition-shifted copy of outT for out-proj row-group pairing
            o2 = post_pool.tile([P, IW], F16, tag="outT2", bufs=2, name="outT2")
            nc.gpsimd.dma_start(o2[DH:P, :], outTs[i][:, :])
            outT2s[i] = o2

        def post_ymm(i, p):
            # paired out-proj: chunk 2p on PE rows 0:64, chunk 2p+1 on 64:128
            ya = ppool.tile([P, D], F32, tag="proj", bufs=2, name="ya")
            yb = ppool.tile([P, D], F32, tag="proj", bufs=2, name="yb")
            nc.tensor.matmul(
                ya[:],
                lhsT=outTs[i][:, ts(2 * p, P)],
                rhs=wo2[0:DH],
                start=True,
                stop=True,
            )
            nc.tensor.matmul(
                yb[:],
                lhsT=outT2s[i][DH:P, ts(2 * p + 1, P)],
                rhs=wo2[DH:P],
                start=True,
                stop=True,
            )
            ysa = yout_pool.tile([P, D], F16, tag="ysb", bufs=4, name="ysa")
            nc.scalar.copy(ysa[:], ya[:])
            nc.sync.dma_start(y[i * (IW // P) + 2 * p], ysa[:])
            return yb

        def post_yb(i, p, yb):
            ysb_ = yout_pool.tile([P, D], F16, tag="ysb", bufs=4, name="ysb")
            nc.vector.tensor_copy(ysb_[:], yb[:])
            nc.sync.dma_start(y[i * (IW // P) + 2 * p + 1], ysb_[:])

        def pend_posts(pending, i):
            ybs = {}

            def mk_ymm(p):
                def f():
                    ybs[p] = post_ymm(i, p)

                return f

            pending.append(lambda: post_rcp(i))
            pending.append(lambda: post_bcast(i))
            pending.append(lambda: post_mul(i, 0))
            pending.append(lambda: post_mul(i, 1))
            pending.append(lambda: post_outT2(i))
            pending.append(mk_ymm(0))
            pending.append(lambda: post_yb(i, 0, ybs[0]))
            pending.append(mk_ymm(1))
            pending.append(lambda: post_yb(i, 1, ybs[1]))

        def emit_post_tail(i):
            # last i-tile: phase-ordered emission (all srows, all rcps, ...)
            # so each in-order engine queue holds every chunk's work for
            # phase k before phase k+1 — per-chunk interleaved emission
            # serializes chunks through the queues instead.
            pv = pvs[i]
            NT = IW // P
            outT = post_pool.tile([DH, IW], F16, tag="outT", bufs=2, name="outTt")
            srows, rcpc, rbc = [], [], []
            for t in range(NT):
                srow = post_pool.tile([1, P], F32, tag="srowc", bufs=4, name="srowc")
                nc.scalar.copy(srow[:], pv[DH : DH + 1, ts(t, P)])
                srows.append(srow)
            for t in range(NT):
                rcp = post_pool.tile([1, P], F32, tag="rcpc", bufs=4, name="rcpc")
                nc.vector.reciprocal_approx_fast(rcp[:], srows[t][:])
                rcpc.append(rcp)
            for t in range(NT):
                rb = post_pool.tile([DH, P], F32, tag="rbc", bufs=4, name="rbc")
                nc.gpsimd.partition_broadcast(rb[:], rcpc[t][:])
                rbc.append(rb)
            for t in range(NT):
                nc.vector.tensor_mul(
                    outT[:, ts(t, P)], pv[0:DH, ts(t, P)], rbc[t][:]
                )
            for t in range(NT):
                # score psum banks are free now so yps alternates tags
                yps = ppool.tile(
                    [P, D],
                    F32,
                    tag="proj" if t % 2 == 0 else "st",
                    bufs=2,
                    name="yps",
                )
                nc.tensor.matmul(
                    yps[:],
                    lhsT=outT[:, ts(t, P)],
                    rhs=wo2[0:DH],
                    start=True,
                    stop=True,
                )
                ysb_ = yout_pool.tile([P, D], F16, tag="ysb", bufs=4, name="ysbt")
                if t % 2 == 0:
                    nc.scalar.copy(ysb_[:], yps[:])
                else:
                    nc.vector.tensor_copy(ysb_[:], yps[:])
                nc.sync.dma_start(y[i * (IW // P) + t], ysb_[:])

        # --- prologue: projections interleaved with i-tiles 0..2.
        # i-tile-0 groups go LAST in each iteration so the kq swap DMAs
        # for proj(i2) have the older tiles' groups as cover. ---
        from collections import deque

        pending = deque()

        def pump():
            if pending:
                pending.popleft()()

        for i2 in range(NI):
            emit_proj_kq(i2)
            emit_proj_v(i2)
            if i2 > 0:
                emit_group(1, 2 * (i2 - 1))
                emit_group(1, 2 * (i2 - 1) + 1)
            if i2 > 1:
                emit_group(2, 2 * (i2 - 2))
                emit_group(2, 2 * (i2 - 2) + 1)
            emit_group(0, 2 * i2)
            emit_group(0, 2 * i2 + 1)
        # trailing groups; the pv backlog is NOT bulk-flushed here — the
        # steady tiles run with a deeper pv lag (8) so the backlog drains
        # interleaved with tile-3's scores instead of as a monolithic
        # Tensor-queue drain that everything pumped afterward waits behind
        emit_group(1, NG - 2)
        emit_group(1, NG - 1)
        for g in range(2 * (NI - 2), NG):
            emit_group(2, g)
        for i in (0, 1, 2):
            pend_posts(pending, i)
        # --- steady state ---
        # Posts for earlier tiles are pumped only once the semaphore their
        # first piece waits on (that tile's pv complete) is satisfied
        # before the piece enters an engine queue: a queued wait parks the
        # whole in-order queue, including later exp halves. The deeper lag
        # (8) also means pv(i,0) — which needs the acc psum slot freed by
        # mul(i-2) — is emitted only at g==8, past that mul's pump slot.
        for i in range(3, NI):
            for g in range(NG):
                emit_group(i, g, lag=8)
                # tile 3 drains the three-tile backlog left by the
                # prologue (27 pieces over 28 slots), two per group
                if i == 3:
                    if g >= 2:
                        pump()
                        pump()
                elif g >= 7:
                    # g >= 7 only: pv(i-1, 15) is flushed inside
                    # emit_group(i, 7) — a post piece emitted before it
                    # would read partial accumulations
                    pump()
            if i < NI - 1:
                pend_posts(pending, i)
        flush_pv(0)
        while pending:
            pump()
        emit_post_tail(NI - 1)
    nc.compile()
    return nc


def _get_nc():
    if "nc" not in _CACHE:
        _CACHE["nc"] = build_bass()
    return _CACHE["nc"]


def _prep_in_maps(x, Wqkv, Wo):
    x = np.asarray(x, dtype=np.float32).reshape(L, D)
    Wqkv = np.asarray(Wqkv, dtype=np.float32)
    Wo = np.asarray(Wo, dtype=np.float32)
    xt = np.ascontiguousarray(x.T).reshape(DC, P, L).astype(np.float16)
    in_maps = []
    for h in range(N_CORES):
        wq = Wqkv[:, 0 * D + h * DH : 0 * D + (h + 1) * DH]
        wk = Wqkv[:, 1 * D + h * DH : 1 * D + (h + 1) * DH]
        wv = Wqkv[:, 2 * D + h * DH : 2 * D + (h + 1) * DH]
        cols = np.concatenate([wq, wq, wk, wk, wv], axis=1)  # [512, 320]
        w_dram = np.ascontiguousarray(cols).reshape(DC, P, WCOLS).astype(np.float16)
        wo_h = np.ascontiguousarray(Wo[h * DH : (h + 1) * DH, :]).astype(np.float16)
        in_maps.append({"xt": xt, "w": w_dram, "wo": wo_h})
    return in_maps


def kernel(x, Wqkv, Wo):
    from concourse import bass_utils

    # zero-egress container: artifact upload is impossible and only feeds
    # trace metadata — replace with a local marker.
    bass_utils.upload_artifacts = lambda tmpdir: f"local://{tmpdir}"

    nc = _get_nc()
    in_maps = _prep_in_maps(x, Wqkv, Wo)
    trace = bool(os.environ.get("KERNEL_TRACE"))
    res = bass_utils.run_bass_kernel_spmd(
        nc, in_maps, core_ids=list(range(N_CORES)), trace=trace
    )
    LAST["exec_time_ns"] = res.exec_time_ns
    LAST["trace"] = res.instructions_and_trace
    acc = np.zeros((L, D), np.float32)
    for r in res.results:
        acc += r["y"].reshape(L, D).astype(np.float32)
    return acc.reshape(1, L, D).astype(np.float32)
